# revision 15
# baseline (speedup 1.0000x reference)
"""Bolmo attention (GQA + QK-RMSNorm + RoPE + causal attention + out-proj)
as an 8-way tensor-parallel Bass kernel for one TRN2 chip.

Sharding: head-parallel. Core c owns Q heads [4c, 4c+4) (wq columns
[256c, 256c+256)), KV head c (wk/wv columns [64c, 64c+64)), and wo rows
[256c, 256c+256). hidden_states is replicated. Each core computes a partial
output (its heads' contribution through wo); the host sums the 8 partials.

v2: fp16 matmul operands throughout (1 cyc/row on the PE, half the DMA/DVE
bytes), PE ones-matmul for the RMS sum-of-squares partials (replaces gpsimd
partition_all_reduce), r_k*scale baked into K so the softmax exp runs with a
uniform scale and can batch 2 key-blocks per activation, exp(s-4) bias to
keep fp16 probabilities in range, DVE mask-multiplies for the causal
diagonal band (precomputed 0/1 tiles), approx-reciprocal + gpsimd broadcast
for the softmax normalization, rope via DMA-shifted row copies (one DVE mul
instead of four), Q heads stored two-up ([128, T] tiles) so rope/rms muls
use all 128 DVE lanes, and the output projection interleaved per 512-token
chunk right after its attention completes to keep the PE dense (HAM at
2.4 GHz).

The QK RMSNorm is over the FULL projected dim (2048 for q, 512 for k), so the
per-core sum-of-squares partials are combined with one tiny on-device
AllReduce ([2, 1024] fp32 per batch).
"""

import os
import sys

import numpy as np

for _p in ("/opt/trn_rl_repo", "/root/.axon_site/_ro/trn_rl_repo"):
    if os.path.isdir(_p) and _p not in sys.path:
        sys.path.insert(0, _p)

from concourse import bacc, masks, mybir, tile  # noqa: E402
from concourse.bass_utils import run_bass_kernel_spmd  # noqa: E402

B, S, H = 2, 1024, 2048
NH, NKV, HD = 32, 8, 64
T = B * S
NCORES = 8
DQ = (NH // NCORES) * HD     # 256 q dims per core
DK = (NKV // NCORES) * HD    # 64 kv dims per core
EPS = 1e-6
SCALE = HD ** -0.5
EXPB = -4.0                  # softmax-invariant exp bias (fp16 range safety)

F32 = mybir.dt.float32
F16 = mybir.dt.float16
AF = mybir.ActivationFunctionType
ALU = mybir.AluOpType

NHT = H // 128      # 16 hidden tiles
NCH = T // 512      # 4 token chunks
SKT = S // 128      # 8 key tiles per batch
VW = 66             # vta block width (64 V + ones + pad, 4B-aligned)


def build(debug=False):
    nc = bacc.Bacc("TRN2", target_bir_lowering=False, debug=False,
                   num_devices=NCORES)

    hs = nc.dram_tensor("hs", [T, H], F16, kind="ExternalInput").ap()
    wq = nc.dram_tensor("wq", [H, DQ], F16, kind="ExternalInput").ap()
    wk = nc.dram_tensor("wk", [H, DK], F16, kind="ExternalInput").ap()
    wv = nc.dram_tensor("wv", [H, DK], F16, kind="ExternalInput").ap()
    wo = nc.dram_tensor("wo", [DQ, H], F16, kind="ExternalInput").ap()
    qnw = nc.dram_tensor("qnw", [DQ], F32, kind="ExternalInput").ap()
    knw = nc.dram_tensor("knw", [DK], F32, kind="ExternalInput").ap()
    cos = nc.dram_tensor("cos", [T, HD], F16, kind="ExternalInput").ap()
    sin = nc.dram_tensor("sin", [T, HD], F16, kind="ExternalInput").ap()
    out = nc.dram_tensor("out", [T, H], F32, kind="ExternalOutput").ap()
    if debug:
        dbg_q = nc.dram_tensor("dbg_q", [DQ, T], F16, kind="ExternalOutput").ap()
        dbg_k = nc.dram_tensor("dbg_k", [128, T], F16, kind="ExternalOutput").ap()
        dbg_ot = nc.dram_tensor("dbg_ot", [DQ, T], F16, kind="ExternalOutput").ap()

    with tile.TileContext(nc) as tc:
        with (
            tc.tile_pool(name="wpool", bufs=1) as wpool,
            tc.tile_pool(name="dram", bufs=1, space="DRAM") as dram,
        ):
            # ---------------- constants & weights ----------------
            idf = wpool.tile([128, 128], F32, tag="idf")
            masks.make_identity(nc, idf[:])
            ident = wpool.tile([128, 128], F16, tag="ident")
            nc.scalar.copy(ident[:], idf[:])
            # identity staged at partition base 64 (for V^T transposes)
            identV = wpool.tile([128, 64], F16, tag="identV")
            nc.scalar.copy(identV[0:64, :], idf[0:64, 0:64])
            nc.scalar.copy(identV[64:128, :], idf[0:64, 0:64])
            onesf = wpool.tile([128, 1], F32, tag="onesf")
            nc.gpsimd.memset(onesf[:], 1.0)
            ones16 = wpool.tile([128, 1], F16, tag="ones16")
            nc.scalar.copy(ones16[:], onesf[:])
            eps1 = wpool.tile([1, 1], F32, tag="eps1")
            nc.gpsimd.memset(eps1[:], EPS)
            expb = wpool.tile([128, 1], F32, tag="expb")
            nc.gpsimd.memset(expb[:], EXPB)
            # k-ssq matmul constant: 256 = (2048/512) * (1/SCALE^2) so both
            # q and k halves share one Sqrt(x/2048 + eps) + recip pipeline
            # (the k eps is effectively eps/64 -- negligible vs msq ~4e-4)
            c256f = wpool.tile([64, 1], F32, tag="c256f")
            nc.gpsimd.memset(c256f[:], 256.0)
            c256 = wpool.tile([64, 1], F16, tag="c256")
            nc.scalar.copy(c256[:], c256f[:])

            qnw_sb = wpool.tile([128, 2], F32, tag="qnw_sb")
            nc.sync.dma_start(qnw_sb[:], qnw.rearrange("(m p) -> p m", p=128))
            kscale = wpool.tile([128, 1], F32, tag="kscale")
            nc.sync.dma_start(kscale[0:64, :], knw.rearrange("(p m) -> p m", m=1))
            nc.gpsimd.memset(kscale[64:128, :], 1.0)

            # prefetch ALL hs chunk tiles first so phase 1 never waits on DMA
            # (weight loads are emitted after and stream in behind them)
            hsball = wpool.tile([128, 16, 4, 512], F16, tag="hsball")
            for c4 in range(NCH):
                for hq in range(4):
                    nc.sync.dma_start(
                        hsball[:, c4 * 4 + hq, :, :],
                        hs[c4 * 512:(c4 + 1) * 512,
                           hq * 512:(hq + 1) * 512]
                        .rearrange("(t p) f -> p t f", p=128))

            wq_sb = wpool.tile([128, NHT * DQ], F16, tag="wq_sb")
            nc.sync.dma_start(
                wq_sb[:].rearrange("p (t m) -> p t m", m=DQ),
                wq.rearrange("(t p) m -> p t m", p=128))
            wkv_sb = wpool.tile([128, NHT * 128], F16, tag="wkv_sb")
            wkv3 = wkv_sb[:].rearrange("p (t c) -> p t c", c=128)
            nc.sync.dma_start(wkv3[:, :, 0:64],
                              wk.rearrange("(t p) m -> p t m", p=128))
            nc.sync.dma_start(wkv3[:, :, 64:128],
                              wv.rearrange("(t p) m -> p t m", p=128))
            wo_sb = wpool.tile([128, 2 * H], F16, tag="wo_sb")

            # causal diagonal-band masks: mask01[i] [128, 1024] covers the
            # ki-pair at relative band position (2i, 2i+1)
            mask01 = [wpool.tile([128, 1024], F16, tag=f"mk{i}",
                                 name=f"mk{i}") for i in range(2)]
            ones_bc = ones16[:].rearrange("p (a m) -> p a m", a=1)
            for i in range(2):
                nc.scalar.copy(mask01[i][:],
                               ones_bc.to_broadcast([128, 1, 1024]))
                nc.gpsimd.affine_select(
                    mask01[i][:].rearrange("p (h f) -> p h f", h=2),
                    mask01[i][:].rearrange("p (h f) -> p h f", h=2),
                    pattern=[[-128, 2], [1, 512]],
                    base=-(2 * i) * 128,
                    channel_multiplier=-1,
                    compare_op=ALU.is_ge,
                    fill=0.0)

            # persistent activations (fp16, feature-major)
            # qa2[m][0:64] = head 2m, [64:128] = head 2m+1
            qa2 = [wpool.tile([128, T], F16, tag=f"qa{m}", name=f"qa{m}")
                   for m in range(2)]
            kv = wpool.tile([128, T], F16, tag="kv")    # K 0:64 (pre-rope), V 64:128
            kk = wpool.tile([128, T], F16, tag="kk")    # roped K duplicated 2-up
            oT = [wpool.tile([128, T], F16, tag=f"oT{m}", name=f"oT{m}")
                  for m in range(2)]
            vta = [wpool.tile([128, SKT * VW], F16, tag=f"vta{b}",
                              name=f"vta{b}") for b in range(2)]

            # ---------------- cos/sin -> feature-major fp16 ----------
            # cosD [128, T]: cos^T duplicated 2-up.  sinD: rows 0:32 =
            # +sin^T[32:64], 32:64 = -sin^T[0:32], duplicated 2-up.
            cosD = wpool.tile([128, T], F16, tag="cosD")
            sinD = wpool.tile([128, T], F16, tag="sinD")
            sinT = wpool.tile([64, T], F16, tag="sinT")
            with tc.tile_pool(name="cs_pool", bufs=2) as cspool, \
                 tc.tile_pool(name="cs_psum", bufs=2, space="PSUM") as cspp:
                for src_, dst in ((cos, cosD[0:64, :]), (sin, sinT[:])):
                    cs_in = cspool.tile([128, (T // 128) * HD], F16,
                                        tag="cs_in")
                    nc.sync.dma_start(
                        cs_in[:].rearrange("p (t d) -> p t d", d=HD),
                        src_.rearrange("(t p) d -> p t d", p=128))
                    for c4 in range(NCH):
                        tp = cspp.tile([64, 512], F16, tag="cs_tp")
                        for j in range(4):
                            tt = c4 * 4 + j
                            nc.tensor.transpose(
                                tp[:, j * 128:(j + 1) * 128],
                                cs_in[:, tt * HD:(tt + 1) * HD],
                                ident[:])
                        nc.scalar.copy(dst[:, c4 * 512:(c4 + 1) * 512], tp[:])
                # qrot rows 0:32 hold x2 (want -x2*sin), rows 32:64 hold x1
                # (want +x1*sin); sin rows 0:32 == 32:64 in the reference emb
                nc.vector.tensor_scalar_mul(sinD[0:32, :], sinT[0:32, :], -1.0)
                nc.vector.tensor_copy(sinD[32:64, :], sinT[32:64, :])
                nc.scalar.copy(cosD[64:128, :], cosD[0:64, :])
                nc.scalar.copy(sinD[64:128, :], sinD[0:64, :])

            # ssq collective rows: cols 0:1024 = q (by chunk), 1024:2048 = k*256
            ccins = [dram.tile([1, 2048], F32, tag=f"cci{p}", name=f"cci{p}")
                     for p in range(2)]
            ccouts = [dram.tile([1, 2048], F32, tag=f"cco{p}", name=f"cco{p}")
                      for p in range(2)]

            # ------------- phase 1: hs^T + proj + ssq + rope, per chunk -----
            def phase1_chunk(c4, w1, pp1):
                cols = slice(c4 * 512, (c4 + 1) * 512)
                pq = [pp1.tile([128, 512], F32, tag=f"pq{m}",
                               name=f"pq{m}_{c4}", bufs=1) for m in range(2)]
                pkv = pp1.tile([128, 512], F32, tag="pkv", bufs=1,
                               name=f"pkv_{c4}")
                for hq in range(4):
                    hsb = hsball[:, c4 * 4 + hq, :, :]
                    for hi in range(4):
                        hh = hq * 4 + hi
                        tp = pp1.tile([128, 512], F16, tag="tp", bufs=2,
                                      name=f"tp{c4}_{hh}")
                        for j in range(4):
                            nc.tensor.transpose(
                                tp[:, j * 128:(j + 1) * 128],
                                hsb[:, j, hi * 128:(hi + 1) * 128],
                                ident[:])
                        hslice = w1.tile([128, 512], F16, tag="hsT", bufs=4,
                                         name=f"hsT_{c4}_{hh}")
                        if hh % 2 == 0:
                            nc.vector.tensor_copy(hslice[:], tp[:])
                        else:
                            nc.scalar.copy(hslice[:], tp[:])
                        for m in range(2):
                            nc.tensor.matmul(
                                pq[m][:],
                                wq_sb[:, hh * DQ + m * 128:
                                      hh * DQ + (m + 1) * 128],
                                hslice[:], start=(hh == 0),
                                stop=(hh == NHT - 1))
                        nc.tensor.matmul(
                            pkv[:], wkv_sb[:, hh * 128:(hh + 1) * 128],
                            hslice[:], start=(hh == 0), stop=(hh == NHT - 1))
                # epilogue: ssq partials via PE ones-matmul; fp16 copies out
                qarP = pp1.tile([1, 512], F32, tag="qar", bufs=1,
                                name=f"qar_{c4}")
                karP = pp1.tile([1, 512], F32, tag="kar", bufs=1,
                                name=f"kar_{c4}")
                for m in range(2):
                    qsq = w1.tile([128, 512], F16, tag="qsq", bufs=2,
                                  name=f"qsq{m}_{c4}")
                    nc.scalar.square(qsq[:], pq[m][:])
                    nc.tensor.matmul(qarP[:], ones16[:, 0:1], qsq[:],
                                     start=(m == 0), stop=(m == 1))
                    nc.scalar.activation(qa2[m][:, cols], pq[m][:], AF.Copy,
                                         scale=qnw_sb[:, m:m + 1])
                ksq = w1.tile([64, 512], F16, tag="ksq", bufs=1,
                              name=f"ksq_{c4}")
                nc.scalar.square(ksq[:], pkv[0:64, :])
                nc.tensor.matmul(karP[:], c256[:, 0:1], ksq[:],
                                 start=True, stop=True)
                nc.scalar.activation(kv[:, cols], pkv[:], AF.Copy,
                                     scale=kscale[:, 0:1])
                # stage ssq rows to SBUF (DMA cannot read PSUM), then DRAM
                qss = w1.tile([1, 512], F32, tag="qss", bufs=2,
                              name=f"qss_{c4}")
                kss = w1.tile([1, 512], F32, tag="kss", bufs=2,
                              name=f"kss_{c4}")
                nc.vector.tensor_copy(qss[:], qarP[:])
                nc.vector.tensor_copy(kss[:], karP[:])
                pair = c4 // 2
                off = (c4 % 2) * 512
                nc.gpsimd.dma_start(ccins[pair][0:1, off:off + 512], qss[:])
                nc.gpsimd.dma_start(ccins[pair][0:1, 1024 + off:1024 + off + 512],
                                    kss[:])

            def rope_chunk(c4, w1):
                cols = slice(c4 * 512, (c4 + 1) * 512)
                for m in range(2):
                    qrot = w1.tile([128, 512], F16, tag="qrot", bufs=2,
                                   name=f"qrot{c4}_{m}")
                    for d0, s0 in ((0, 32), (32, 0), (64, 96), (96, 64)):
                        nc.sync.dma_start(qrot[d0:d0 + 32, :],
                                          qa2[m][s0:s0 + 32, cols])
                    t2 = w1.tile([128, 512], F16, tag="t2", bufs=2,
                                 name=f"t2_{c4}_{m}")
                    t3 = w1.tile([128, 512], F16, tag="t3", bufs=2,
                                 name=f"t3_{c4}_{m}")
                    nc.vector.tensor_mul(t2[:], qa2[m][:, cols], cosD[:, cols])
                    nc.vector.tensor_mul(t3[:], qrot[:], sinD[:, cols])
                    nc.vector.tensor_add(qa2[m][:, cols], t2[:], t3[:])
                krot = w1.tile([64, 512], F16, tag="krot", bufs=1,
                               name=f"krot{c4}")
                nc.sync.dma_start(krot[0:32, :], kv[32:64, cols])
                nc.sync.dma_start(krot[32:64, :], kv[0:32, cols])
                t2k = w1.tile([64, 512], F16, tag="t2k", bufs=1,
                              name=f"t2k{c4}")
                t3k = w1.tile([64, 512], F16, tag="t3k", bufs=1,
                              name=f"t3k{c4}")
                nc.vector.tensor_mul(t2k[:], kv[0:64, cols], cosD[0:64, cols])
                nc.vector.tensor_mul(t3k[:], krot[:], sinD[0:64, cols])
                nc.vector.tensor_add(kk[0:64, cols], t2k[:], t3k[:])
                nc.scalar.copy(kk[64:128, cols], kk[0:64, cols])

            def ssq_collective(pair):
                nc.gpsimd.collective_compute(
                    "AllReduce", ALU.add,
                    ins=[ccins[pair].opt()], outs=[ccouts[pair].opt()],
                    replica_groups=[list(range(NCORES))],
                )

            def batch_prep(b):
                # rms factors; fold r_q into qa2, r_k*SCALE into kk.
                # rr cols 0:1024 = q ssq, 1024:2048 = 256*k ssq, so one
                # uniform Sqrt(x/2048+eps) + recip yields r_q | r_k*SCALE.
                bcols = slice(b * S, (b + 1) * S)
                rr = wpool.tile([1, 2048], F32, tag=f"rr{b}", name=f"rr{b}")
                nc.gpsimd.dma_start(rr[:], ccouts[b][:])
                nc.scalar.activation(rr[:], rr[:], AF.Sqrt,
                                     bias=eps1[:, 0:1],
                                     scale=1.0 / (NH * HD))
                rri = wpool.tile([1, 2048], F32, tag=f"rri{b}",
                                 name=f"rri{b}")
                nc.vector.reciprocal_approx_fast(rri[:], rr[:])
                r16 = wpool.tile([1, 2048], F16, tag=f"r16{b}",
                                 name=f"r16{b}")
                nc.vector.tensor_copy(r16[:], rri[:])
                rqb = wpool.tile([128, 1024], F16, tag=f"rqb{b}",
                                 name=f"rqb{b}")
                nc.gpsimd.partition_broadcast(rqb[:], r16[0:1, 0:1024])
                rkb = wpool.tile([128, 1024], F16, tag=f"rkb{b}",
                                 name=f"rkb{b}")
                nc.gpsimd.partition_broadcast(rkb[:], r16[0:1, 1024:2048])
                for m in range(2):
                    nc.vector.tensor_mul(qa2[m][:, bcols], qa2[m][:, bcols],
                                         rqb[:])
                nc.vector.tensor_mul(kk[:, bcols], kk[:, bcols], rkb[:])

            def vta_batch(b, vpp):
                nc.gpsimd.memset(vta[b][:], 0.0)
                vtp = vpp.tile([128, 512], F16, tag="tp", bufs=2,
                               name=f"vtp{b}")
                for ki in range(SKT):
                    nc.tensor.transpose(
                        vtp[:, ki * 64:ki * 64 + 64],
                        kv[64:128, b * S + ki * 128:b * S + (ki + 1) * 128],
                        identV[64:128, :])
                for ki in range(SKT):
                    nc.scalar.copy(vta[b][:, ki * VW:ki * VW + 64],
                                   vtp[:, ki * 64:(ki + 1) * 64])
                nc.scalar.copy(
                    vta[b][:].rearrange("p (k c) -> p k c", c=VW)[:, :, 64:65],
                    ones_bc.to_broadcast([128, SKT, 1]))

            def attention_chunk(b, qj, apool, pstp, povp):
                # scores + softmax + AV for the 512-query chunk (b, qj)
                boff = b * S
                qc0 = boff + qj * 512
                nkt = 4 * (qj + 1)
                for pbase, heads in ((0, (0, 2)), (64, (1, 3))):
                    ovps = {}
                    for h in heads:
                        ovps[h] = povp.tile([VW, 512], F32, tag="ovp",
                                            bufs=2, name=f"ovp{b}{qj}{h}")
                    for pi in range(nkt // 2):
                        kia, kib = 2 * pi, 2 * pi + 1
                        stps = {h: pstp.tile([128, 1024], F32, tag="stp",
                                             bufs=2,
                                             name=f"stp{b}{qj}{pbase}{pi}{h}")
                                for h in heads}
                        for half, kii in ((0, kia), (1, kib)):
                            for h in heads:
                                nc.tensor.matmul(
                                    stps[h][:, half * 512:(half + 1) * 512],
                                    kk[pbase:pbase + 64,
                                       boff + kii * 128:boff + (kii + 1) * 128],
                                    qa2[h // 2][pbase:pbase + 64,
                                                qc0:qc0 + 512],
                                    start=True, stop=True)
                        pts = {}
                        for h in heads:
                            pt = apool.tile([128, 1024], F16, tag="pt",
                                            bufs=4,
                                            name=f"pt{b}{qj}{pbase}{pi}{h}")
                            nc.scalar.activation(pt[:], stps[h][:], AF.Exp,
                                                 bias=expb[:, 0:1])
                            if kia >= 4 * qj:
                                mi = (kia - 4 * qj) // 2
                                nc.vector.tensor_mul(pt[:], pt[:],
                                                     mask01[mi][:])
                            pts[h] = pt
                        for half, kii in ((0, kia), (1, kib)):
                            for h in heads:
                                nc.tensor.matmul(
                                    ovps[h][:],
                                    vta[b][:, kii * VW:(kii + 1) * VW],
                                    pts[h][:, half * 512:(half + 1) * 512],
                                    start=(kii == 0), stop=(kii == nkt - 1))
                    # softmax normalization: 1/l via approx recip, gpsimd
                    # broadcast, fused into the PSUM->oT drain
                    for h in heads:
                        lg = apool.tile([1, 512], F32, tag="lg", bufs=2,
                                        name=f"lg{b}{qj}{h}")
                        nc.vector.tensor_copy(lg[:], ovps[h][64:65, :])
                        li = apool.tile([1, 512], F32, tag="li", bufs=2,
                                        name=f"li{b}{qj}{h}")
                        nc.vector.reciprocal_approx_fast(li[:], lg[:])
                        lb = apool.tile([64, 512], F32, tag="lb", bufs=2,
                                        name=f"lb{b}{qj}{h}")
                        nc.gpsimd.partition_broadcast(lb[:], li[:])
                        m, prow = h // 2, (h % 2) * 64
                        nc.vector.tensor_mul(
                            oT[m][prow:prow + 64, qc0:qc0 + 512],
                            ovps[h][0:64, :], lb[:])

            def wo_chunk(b, qj, wpool4, ppo):
                for tt in range(b * 8 + qj * 4, b * 8 + qj * 4 + 4):
                    for nj in range(4):
                        po = ppo.tile([128, 512], F32, tag="po", bufs=2,
                                      name=f"po{tt}_{nj}")
                        for m in range(2):
                            nc.tensor.matmul(
                                po[:],
                                oT[m][:, tt * 128:(tt + 1) * 128],
                                wo_sb[:, m * H + nj * 512:
                                      m * H + (nj + 1) * 512],
                                start=(m == 0), stop=(m == 1))
                        outc = wpool4.tile([128, 512], F32, tag="outc",
                                           bufs=4, name=f"outc{tt}_{nj}")
                        if nj % 2 == 0:
                            nc.scalar.copy(outc[:], po[:])
                        else:
                            nc.vector.tensor_copy(outc[:], po[:])
                        nc.sync.dma_start(
                            out[tt * 128:(tt + 1) * 128,
                                nj * 512:(nj + 1) * 512], outc[:])

            # ---------------- orchestration ----------------
            with tc.tile_pool(name="w1_pool", bufs=2) as w1, \
                 tc.tile_pool(name="p1_psum", bufs=1, space="PSUM") as pp1:
                phase1_chunk(0, w1, pp1)
                rope_chunk(0, w1)
                phase1_chunk(1, w1, pp1)
                rope_chunk(1, w1)
                ssq_collective(0)
                vta_batch(0, pp1)
                phase1_chunk(2, w1, pp1)
                rope_chunk(2, w1)
                phase1_chunk(3, w1, pp1)
                rope_chunk(3, w1)
                ssq_collective(1)
                batch_prep(0)
                vta_batch(1, pp1)
                # out-proj weights only needed from the attention phase on
                for m in range(2):
                    nc.sync.dma_start(wo_sb[:, m * H:(m + 1) * H],
                                      wo[m * 128:(m + 1) * 128, :])

            with tc.tile_pool(name="a_pool", bufs=2) as apool, \
                 tc.tile_pool(name="w4_pool", bufs=2) as wpool4, \
                 tc.tile_pool(name="stp_psum", bufs=1, space="PSUM") as pstp, \
                 tc.tile_pool(name="ovp_psum", bufs=1, space="PSUM") as povp, \
                 tc.tile_pool(name="po_psum", bufs=1, space="PSUM") as ppo:
                attention_chunk(0, 0, apool, pstp, povp)
                batch_prep(1)
                attention_chunk(0, 1, apool, pstp, povp)
                wo_chunk(0, 0, wpool4, ppo)
                attention_chunk(1, 0, apool, pstp, povp)
                wo_chunk(0, 1, wpool4, ppo)
                attention_chunk(1, 1, apool, pstp, povp)
                wo_chunk(1, 0, wpool4, ppo)
                wo_chunk(1, 1, wpool4, ppo)

            if debug:
                for m in range(2):
                    nc.sync.dma_start(dbg_q[m * 128:(m + 1) * 128, :],
                                      qa2[m][:])
                    nc.sync.dma_start(dbg_ot[m * 128:(m + 1) * 128, :],
                                      oT[m][:])
                nc.sync.dma_start(dbg_k[:], kk[:])
    nc.compile()
    return nc


_CACHED = {}


def _get_nc(debug=False):
    if debug not in _CACHED:
        _CACHED[debug] = build(debug)
    return _CACHED[debug]


def _is_causal_mask(mask):
    m = np.asarray(mask)
    if m.shape != (B, 1, S, S):
        return False
    tri = np.tril(np.ones((S, S), dtype=bool))
    for b in range(B):
        mb = m[b, 0]
        if not np.all(mb[tri] == 0.0):
            return False
        if not np.all(mb[~tri] <= -1e8):
            return False
    return True


def _numpy_fallback(hidden_states, cos, sin, attention_mask, wq, wk, wv, wo,
                    q_norm_w, k_norm_w):
    hs = np.asarray(hidden_states, np.float64)
    b, s, _ = hs.shape
    g = NH // NKV

    def rms(x, w):
        var = np.mean(x * x, axis=-1, keepdims=True)
        return w * (x / np.sqrt(var + EPS))

    def rot(x):
        x1, x2 = np.split(x, 2, axis=-1)
        return np.concatenate((-x2, x1), axis=-1)

    q = rms(hs @ np.asarray(wq, np.float64), np.asarray(q_norm_w, np.float64))
    k = rms(hs @ np.asarray(wk, np.float64), np.asarray(k_norm_w, np.float64))
    v = hs @ np.asarray(wv, np.float64)
    q = q.reshape(b, s, NH, HD).transpose(0, 2, 1, 3)
    k = k.reshape(b, s, NKV, HD).transpose(0, 2, 1, 3)
    v = v.reshape(b, s, NKV, HD).transpose(0, 2, 1, 3)
    c = np.asarray(cos, np.float64)[:, None]
    sn = np.asarray(sin, np.float64)[:, None]
    q = q * c + rot(q) * sn
    k = k * c + rot(k) * sn
    k = np.repeat(k, g, axis=1)
    v = np.repeat(v, g, axis=1)
    sc = np.einsum('bhqd,bhkd->bhqk', q, k) * SCALE + np.asarray(
        attention_mask, np.float64)
    sc = sc - sc.max(axis=-1, keepdims=True)
    e = np.exp(sc)
    attn = e / e.sum(axis=-1, keepdims=True)
    o = np.einsum('bhqk,bhkd->bhqd', attn, v)
    o = o.transpose(0, 2, 1, 3).reshape(b, s, NH * HD)
    return (o @ np.asarray(wo, np.float64)).astype(np.float32)


def make_in_maps(hidden_states, cos, sin, wq, wk, wv, wo, q_norm_w, k_norm_w):
    hsf = np.ascontiguousarray(
        np.asarray(hidden_states).reshape(T, H).astype(np.float16))
    cosf = np.ascontiguousarray(
        np.asarray(cos).reshape(T, HD).astype(np.float16))
    sinf = np.ascontiguousarray(
        np.asarray(sin).reshape(T, HD).astype(np.float16))
    wqf = np.asarray(wq).astype(np.float16)
    wkf = np.asarray(wk).astype(np.float16)
    wvf = np.asarray(wv).astype(np.float16)
    wof = np.asarray(wo).astype(np.float16)
    in_maps = []
    for c in range(NCORES):
        qs = slice(c * DQ, (c + 1) * DQ)
        ks = slice(c * DK, (c + 1) * DK)
        in_maps.append({
            "hs": hsf,
            "wq": np.ascontiguousarray(wqf[:, qs]),
            "wk": np.ascontiguousarray(wkf[:, ks]),
            "wv": np.ascontiguousarray(wvf[:, ks]),
            "wo": np.ascontiguousarray(wof[qs, :]),
            "qnw": np.ascontiguousarray(np.asarray(q_norm_w, np.float32)[qs]),
            "knw": np.ascontiguousarray(np.asarray(k_norm_w, np.float32)[ks]),
            "cos": cosf,
            "sin": sinf,
        })
    return in_maps


def run(inputs, debug=False, trace=False):
    nc = _get_nc(debug)
    in_maps = make_in_maps(
        inputs["hidden_states"], inputs["cos"], inputs["sin"],
        inputs["wq"], inputs["wk"], inputs["wv"], inputs["wo"],
        inputs["q_norm_w"], inputs["k_norm_w"])
    return run_bass_kernel_spmd(nc, in_maps, list(range(NCORES)), trace=trace)


def kernel(hidden_states, cos, sin, attention_mask, wq, wk, wv, wo,
           q_norm_w, k_norm_w):
    if not _is_causal_mask(attention_mask):
        return _numpy_fallback(hidden_states, cos, sin, attention_mask,
                               wq, wk, wv, wo, q_norm_w, k_norm_w)
    res = run({"hidden_states": hidden_states, "cos": cos, "sin": sin,
               "wq": wq, "wk": wk, "wv": wv, "wo": wo,
               "q_norm_w": q_norm_w, "k_norm_w": k_norm_w})
    total = np.zeros((T, H), np.float64)
    for c in range(NCORES):
        total += res.results[c]["out"].astype(np.float64)
    return total.reshape(B, S, H).astype(np.float32)
